# revision 1
# baseline (speedup 1.0000x reference)
"""BertLexer Trainium2 kernel.

Computes, for full inputs
    word_indices [16,256] int, span_start/span_end [16,256] int,
    W_embed [50002,256] f32, hidden_states [12,16,512,768] f32
the reference
    word_emb = W_embed[word_indices]                                # [B,W,E]
    bert_sub = hidden_states.mean(axis=0)                           # [B,S,H]
    bert_emb[b,w] = mean(bert_sub[b, span_start:span_end])          # [B,W,H]
    out = concat([word_emb, bert_emb], axis=2)                      # [B,W,E+H]

Strategy: data-parallel over the batch dim across 8 NeuronCores (2 batches
per core).  Per core, the 12-layer mean and the ragged span-mean are fused
into one PE matmul chain: build a span-selection matrix
M[w,s] = (start_w <= s < end_w) / (12*len_w) on-chip, transpose it with the
PE, then accumulate  out_bert = sum_l M @ h[l]  into PSUM with fp32r
matmuls.  Word embeddings are row-gathered from DRAM with indirect DMA.
The kernel is HBM-bound on the hidden_states read (~38MB/core).
"""

import sys

import numpy as np

if "/opt/trn_rl_repo" not in sys.path:
    sys.path.insert(0, "/opt/trn_rl_repo")

import concourse.bacc as bacc
import concourse.bass as bass
import concourse.mybir as mybir
import concourse.tile as tile
from concourse.masks import make_identity

B, W, S, H, L, E, V = 16, 256, 512, 768, 12, 256, 50002
NCORES = 8
BPC = B // NCORES  # batches per core
P = 128
WT = W // P  # word-index tiles per batch
ST = S // P  # subword (contraction) tiles per batch
NCHUNKS = [(0, 512), (512, 256)]  # PSUM-bank-sized pieces of H

F32 = mybir.dt.float32
F32R = mybir.dt.float32r
I32 = mybir.dt.int32


def build_program(reps=1):
    nc = bacc.Bacc(
        "TRN2", target_bir_lowering=False, debug=False, num_devices=NCORES
    )
    wi = nc.dram_tensor("word_indices", [BPC, W], I32, kind="ExternalInput").ap()
    ss = nc.dram_tensor("span_start", [BPC, W], I32, kind="ExternalInput").ap()
    se = nc.dram_tensor("span_end", [BPC, W], I32, kind="ExternalInput").ap()
    emb = nc.dram_tensor("W_embed", [V, E], F32, kind="ExternalInput").ap()
    hs = nc.dram_tensor("hidden_states", [L, BPC, S, H], F32, kind="ExternalInput").ap()
    out = nc.dram_tensor("out", [BPC, W, E + H], F32, kind="ExternalOutput").ap()

    with tile.TileContext(nc) as tc:
        with (
            tc.tile_pool(name="const", bufs=1) as const_pool,
            tc.tile_pool(name="idx", bufs=2) as idx_pool,
            tc.tile_pool(name="mask", bufs=2) as mask_pool,
            tc.tile_pool(name="maskT", bufs=2) as maskT_pool,
            tc.tile_pool(name="hbuf", bufs=8) as h_pool,
            tc.tile_pool(name="hsum", bufs=8) as hsum_pool,
            tc.tile_pool(name="obuf", bufs=2) as o_pool,
            tc.tile_pool(name="ptr", bufs=2, space="PSUM") as ptr_pool,
            tc.tile_pool(name="pout", bufs=1, space="PSUM") as pout_pool,
        ):
            identity = const_pool.tile([P, P], F32)
            make_identity(nc, identity)
            iota_i = const_pool.tile([P, S], I32)
            nc.gpsimd.iota(iota_i, pattern=[[1, S]], base=0, channel_multiplier=0)
            iota_f = const_pool.tile([P, S], F32)
            nc.gpsimd.tensor_copy(iota_f, iota_i)

            for rep in range(reps):
              for b in range(BPC):
                # --- index loads on the ACT queue (tiny), h DMAs on the sync
                # queue: one 1.57MB DMA per layer via the s = c*128+p
                # interleave, which lands tile[:, c*H:(c+1)*H] = h[l, c*128+p, :]
                # so each column block is exactly the matmul rhs for s-tile c ---
                idx_tiles = []
                for wt in range(WT):
                    wsl = slice(wt * P, (wt + 1) * P)
                    ss_i = idx_pool.tile([P, 1], I32, tag="ss_i")
                    se_i = idx_pool.tile([P, 1], I32, tag="se_i")
                    wi_i = idx_pool.tile([P, 1], I32, tag="wi_i", bufs=2 * WT)
                    nc.scalar.dma_start(out=ss_i, in_=ss[b, wsl, None])
                    nc.scalar.dma_start(out=se_i, in_=se[b, wsl, None])
                    nc.scalar.dma_start(out=wi_i, in_=wi[b, wsl, None])
                    idx_tiles.append((ss_i, se_i, wi_i))
                STA = ST - 1  # s-tiles 0..2 stream first; s-tile 3 last
                hts_a, hts_b = [], []
                for l in range(L):
                    ha = h_pool.tile([P, STA * H], F32, tag="ha", name=f"ha_{b}_{l}")
                    src = hs[l, b, 0 : STA * P, :].rearrange("(c p) h -> p c h", p=P)
                    nc.sync.dma_start(
                        out=ha.rearrange("p (c h) -> p c h", h=H), in_=src
                    )
                    hts_a.append(ha)
                for l in range(L):
                    hb = h_pool.tile([P, H], F32, tag="hb", name=f"hb_{b}_{l}")
                    nc.sync.dma_start(out=hb, in_=hs[l, b, STA * P : S, :])
                    hts_b.append(hb)

                # --- span-selection matrix, transposed: [s, w] ---
                maskT_all = maskT_pool.tile([P, ST * W], F32, tag="maskT")
                wi_tiles = []
                for wt in range(WT):
                    ss_i, se_i, wi_i = idx_tiles[wt]
                    wi_tiles.append(wi_i)
                    ss_f = idx_pool.tile([P, 1], F32, tag="ss_f")
                    se_f = idx_pool.tile([P, 1], F32, tag="se_f")
                    nc.vector.tensor_copy(ss_f, ss_i)
                    nc.vector.tensor_copy(se_f, se_i)
                    len_f = idx_pool.tile([P, 1], F32, tag="len_f")
                    nc.vector.tensor_tensor(
                        len_f, se_f, ss_f, op=mybir.AluOpType.subtract
                    )
                    rlen = idx_pool.tile([P, 1], F32, tag="rlen")
                    nc.vector.reciprocal(rlen, len_f)
                    scale = idx_pool.tile([P, 1], F32, tag="scale")
                    nc.vector.tensor_scalar_mul(scale, rlen, 1.0 / L)
                    # m1 = (iota >= start) * scale ; m2 = (iota < end)
                    m1 = mask_pool.tile([P, S], F32, tag="m1")
                    nc.vector.tensor_scalar(
                        m1,
                        iota_f,
                        scalar1=ss_f[:, :1],
                        scalar2=scale[:, :1],
                        op0=mybir.AluOpType.is_ge,
                        op1=mybir.AluOpType.mult,
                    )
                    m2 = mask_pool.tile([P, S], F32, tag="m2")
                    nc.vector.tensor_scalar(
                        m2,
                        iota_f,
                        scalar1=se_f[:, :1],
                        scalar2=None,
                        op0=mybir.AluOpType.is_lt,
                    )
                    mM = mask_pool.tile([P, S], F32, tag="mM")
                    nc.vector.tensor_tensor(mM, m1, m2, op=mybir.AluOpType.mult)
                    for st in range(ST):
                        ptr = ptr_pool.tile([P, P], F32, space="PSUM", tag="ptr")
                        nc.tensor.transpose(ptr, mM[:, st * P : (st + 1) * P], identity)
                        col = st * W + wt * P
                        nc.scalar.copy(maskT_all[:, col : col + P], ptr)

                # --- fused layer-mean + span-mean:  pout[wt] = sum_l M @ h[l] ---
                pouts = [
                    pout_pool.tile(
                        [P, H], F32, space="PSUM", tag=f"pout{wt}", name=f"pout{wt}_{b}"
                    )
                    for wt in range(WT)
                ]
                # --- assemble each output row tile [wemb | bert] in SBUF and
                # store once per w-tile (512KB stores avoid the slow small-DMA
                # queue).  Word-embedding gather lands directly in the tile ---
                obufs = []
                for wt in range(WT):
                    obuf = o_pool.tile(
                        [P, E + H], F32, tag="obuf", name=f"obuf_{b}_{wt}"
                    )
                    nc.gpsimd.indirect_dma_start(
                        out=obuf[:, 0:E],
                        out_offset=None,
                        in_=emb[:, :],
                        in_offset=bass.IndirectOffsetOnAxis(
                            ap=wi_tiles[wt][:, :1], axis=0
                        ),
                    )
                    obufs.append(obuf)

                # --- exact f32 12-layer sum on DVE, l-major within each DMA
                # group so chains track arrivals; each s-tile's span matmuls
                # fire as soon as its sum is final (st0-2 mid-stream) ---
                def emit_matmuls(st, hsum_st):
                    first = st == 0
                    last = st == ST - 1
                    for wt in range(WT):
                        col = st * W + wt * P
                        lh = maskT_all[:, col : col + P]
                        for n0, nl in NCHUNKS:
                            nc.tensor.matmul(
                                pouts[wt][:, n0 : n0 + nl],
                                lhsT=lh,
                                rhs=hsum_st[:, n0 : n0 + nl],
                                start=first,
                                stop=last,
                            )

                hsums = [
                    hsum_pool.tile([P, H], F32, tag="hsum", name=f"hsum_{b}_{st}")
                    for st in range(ST)
                ]
                for st in range(STA):
                    hsl = slice(st * H, (st + 1) * H)
                    nc.vector.tensor_tensor(
                        hsums[st], hts_a[0][:, hsl], hts_a[1][:, hsl],
                        op=mybir.AluOpType.add,
                    )
                for l in range(2, L):
                    for st in range(STA):
                        hsl = slice(st * H, (st + 1) * H)
                        nc.vector.tensor_tensor(
                            hsums[st], hsums[st], hts_a[l][:, hsl],
                            op=mybir.AluOpType.add,
                        )
                for st in range(STA):
                    emit_matmuls(st, hsums[st])
                st3 = ST - 1
                nc.vector.tensor_tensor(
                    hsums[st3], hts_b[0], hts_b[1], op=mybir.AluOpType.add
                )
                for l in range(2, L):
                    nc.vector.tensor_tensor(
                        hsums[st3], hsums[st3], hts_b[l], op=mybir.AluOpType.add
                    )
                emit_matmuls(st3, hsums[st3])

                # --- per-region PSUM copy into the row tile, then one store
                # per w-tile.  Last batch's stores ride the idle sync queue ---
                store_eng = (
                    nc.sync if (rep == reps - 1 and b == BPC - 1) else nc.scalar
                )
                for wt in range(WT):
                    wsl = slice(wt * P, (wt + 1) * P)
                    for n0, nl in NCHUNKS:
                        nc.vector.tensor_copy(
                            obufs[wt][:, E + n0 : E + n0 + nl],
                            pouts[wt][:, n0 : n0 + nl],
                        )
                    store_eng.dma_start(out=out[b, wsl, :], in_=obufs[wt])

    nc.compile()
    return nc


_NC = None


def _get_program():
    global _NC
    if _NC is None:
        _NC = build_program()
    return _NC


def make_in_maps(word_indices, span_start, span_end, W_embed, hidden_states):
    emb = np.ascontiguousarray(W_embed, dtype=np.float32)
    in_maps = []
    for c in range(NCORES):
        bsl = slice(BPC * c, BPC * (c + 1))
        in_maps.append(
            {
                "word_indices": np.ascontiguousarray(
                    word_indices[bsl], dtype=np.int32
                ),
                "span_start": np.ascontiguousarray(span_start[bsl], dtype=np.int32),
                "span_end": np.ascontiguousarray(span_end[bsl], dtype=np.int32),
                "W_embed": emb,
                "hidden_states": np.ascontiguousarray(
                    hidden_states[:, bsl], dtype=np.float32
                ),
            }
        )
    return in_maps


def run(word_indices, span_start, span_end, W_embed, hidden_states, **run_kwargs):
    from concourse.bass_utils import run_bass_kernel_spmd

    nc = _get_program()
    in_maps = make_in_maps(word_indices, span_start, span_end, W_embed, hidden_states)
    res = run_bass_kernel_spmd(nc, in_maps, core_ids=list(range(NCORES)), **run_kwargs)
    out = np.concatenate([res.results[c]["out"] for c in range(NCORES)], axis=0)
    return out, res


def kernel(word_indices, span_start, span_end, W_embed, hidden_states):
    out, _ = run(word_indices, span_start, span_end, W_embed, hidden_states)
    return out



# revision 7
# speedup vs baseline: 1.0845x; 1.0845x over previous
"""BertLexer Trainium2 kernel.

Computes, for full inputs
    word_indices [16,256] int, span_start/span_end [16,256] int,
    W_embed [50002,256] f32, hidden_states [12,16,512,768] f32
the reference
    word_emb = W_embed[word_indices]                                # [B,W,E]
    bert_sub = hidden_states.mean(axis=0)                           # [B,S,H]
    bert_emb[b,w] = mean(bert_sub[b, span_start:span_end])          # [B,W,H]
    out = concat([word_emb, bert_emb], axis=2)                      # [B,W,E+H]

Strategy: data-parallel over the batch dim across 8 NeuronCores (2 batches
per core).  Only subwords below max(span_end) are ever referenced, so the
host slices hidden_states to SP = 384+T rows per batch before staging.
Per (layer, batch) the kernel issues one flat DMA [128, 2304] whose
partition p holds subwords 3p..3p+2 (9216B contiguous runs) plus a packed
tail DMA [128, 6T] holding subwords 384..SP flattened.  The 12-layer sum
runs as two interleaved DVE chains (main 2304 cols, tail 6T cols); the
packed tail sum is un-flattened to [T, 768] by a small SBUF->SBUF DMA and
contracted by a T-partition matmul that opens the PSUM accumulation while
the main stream is still arriving.  Span selection uses an on-chip mask
M[w,s] = (start_w <= s < end_w)/(12*len_w) built against a layout-matched
iota, PE-transposed, and contracted with fp32r matmuls.  Word embeddings
are row-gathered from DRAM with indirect DMA into the output row tile;
one 512KB store per w-tile.  HBM-bound on the hidden_states read.
"""

import sys

import numpy as np

if "/opt/trn_rl_repo" not in sys.path:
    sys.path.insert(0, "/opt/trn_rl_repo")

import concourse.bacc as bacc
import concourse.bass as bass
import concourse.mybir as mybir
import concourse.tile as tile
from concourse.masks import make_identity

B, W, S, H, L, E, V = 16, 256, 512, 768, 12, 256, 50002
NCORES = 8
BPC = B // NCORES  # batches per core
P = 128
WT = W // P  # word-index tiles per batch
SFULL = 3 * P  # subwords covered by the full-region tiles (s = 3p + j)
CF = 3 * H  # full-region tile cols (2304)
NCHUNKS = [(0, 512), (512, 256)]  # PSUM-bank-sized pieces of H

F32 = mybir.dt.float32
I32 = mybir.dt.int32


def build_program(T):
    """T = tail subword count (power of two <= 128, or 0). SP = 384 + T."""
    SP = SFULL + T
    CT = (T * H) // P  # packed tail cols (6T)
    CW = CF + CT
    SPM = SFULL + T  # mask columns
    nc = bacc.Bacc(
        "TRN2", target_bir_lowering=False, debug=False, num_devices=NCORES
    )
    wi = nc.dram_tensor("word_indices", [BPC, W], I32, kind="ExternalInput").ap()
    ss = nc.dram_tensor("span_start", [BPC, W], I32, kind="ExternalInput").ap()
    se = nc.dram_tensor("span_end", [BPC, W], I32, kind="ExternalInput").ap()
    emb = nc.dram_tensor("W_embed", [V, E], F32, kind="ExternalInput").ap()
    hs = nc.dram_tensor("hidden_states", [L, BPC, SP * H], F32, kind="ExternalInput").ap()
    out = nc.dram_tensor("out", [BPC, W, E + H], F32, kind="ExternalOutput").ap()
    tsc = (
        nc.dram_tensor("tail_scratch", [BPC, T * H], F32, kind="Internal").ap()
        if T
        else None
    )

    with tile.TileContext(nc) as tc:
        with (
            tc.tile_pool(name="const", bufs=1) as const_pool,
            tc.tile_pool(name="idx", bufs=2) as idx_pool,
            tc.tile_pool(name="mask", bufs=2) as mask_pool,
            tc.tile_pool(name="maskT", bufs=2) as maskT_pool,
            tc.tile_pool(name="hbuf", bufs=12) as h_pool,
            tc.tile_pool(name="hsum", bufs=2) as hsum_pool,
            tc.tile_pool(name="tailb", bufs=2) as tail_pool,
            tc.tile_pool(name="obuf", bufs=4) as o_pool,
            tc.tile_pool(name="ptr", bufs=2, space="PSUM") as ptr_pool,
            tc.tile_pool(name="pout", bufs=1, space="PSUM") as pout_pool,
        ):
            identity = const_pool.tile([P, P], F32)
            make_identity(nc, identity)
            # iota column c holds the subword index mapped to mask column c:
            # cols j*128+p (j<3) -> 3p+j; cols 384.. -> 384..SP-1 (tail).
            iota_i = const_pool.tile([P, SPM], I32)
            nc.gpsimd.iota(
                iota_i[:, 0:SFULL], pattern=[[1, 3], [3, P]], base=0,
                channel_multiplier=0,
            )
            if T:
                nc.gpsimd.iota(
                    iota_i[:, SFULL:SPM], pattern=[[1, T]], base=SFULL,
                    channel_multiplier=0,
                )
            iota_f = const_pool.tile([P, SPM], F32)
            nc.gpsimd.tensor_copy(iota_f, iota_i)

            # --- all index loads up front on the ACT queue ---
            idx_tiles = {}
            for b in range(BPC):
                for wt in range(WT):
                    wsl = slice(wt * P, (wt + 1) * P)
                    ss_i = idx_pool.tile([P, 1], I32, tag="ss_i", bufs=2 * WT)
                    se_i = idx_pool.tile([P, 1], I32, tag="se_i", bufs=2 * WT)
                    wi_i = idx_pool.tile([P, 1], I32, tag="wi_i", bufs=2 * WT)
                    nc.scalar.dma_start(out=ss_i, in_=ss[b, wsl, None])
                    nc.scalar.dma_start(out=se_i, in_=se[b, wsl, None])
                    nc.scalar.dma_start(out=wi_i, in_=wi[b, wsl, None])
                    idx_tiles[(b, wt)] = (ss_i, se_i, wi_i)

            for b in range(BPC):
                # --- h DMAs: packed tail on the gpsimd queue (lands early),
                # big flat [128, 2304] (9216B runs) on the sync queue ---
                h_tiles = []
                for l in range(L):
                    ht = h_pool.tile([P, CW], F32, tag="h", name=f"h_{b}_{l}")
                    if T:
                        nc.gpsimd.dma_start(
                            out=ht[:, CF:CW],
                            in_=hs[l, b, SFULL * H : SP * H].rearrange(
                                "(p x) -> p x", p=P
                            ),
                        )
                    nc.sync.dma_start(
                        out=ht[:, 0:CF],
                        in_=hs[l, b, 0 : SFULL * H].rearrange("(p x) -> p x", p=P),
                    )
                    h_tiles.append(ht)

                # --- span-selection masks + PE transposes ---
                maskT_full = maskT_pool.tile([P, 3 * W], F32, tag="mtf")
                maskT_tail = None
                if T:
                    maskT_tail = maskT_pool.tile([T, W], F32, tag="mtt", name=f"mtt_{b}")
                wi_list = []
                for wt in range(WT):
                    ss_i, se_i, wi_i = idx_tiles[(b, wt)]
                    wi_list.append(wi_i)
                    ss_f = idx_pool.tile([P, 1], F32, tag="ss_f")
                    se_f = idx_pool.tile([P, 1], F32, tag="se_f")
                    nc.vector.tensor_copy(ss_f, ss_i)
                    nc.vector.tensor_copy(se_f, se_i)
                    len_f = idx_pool.tile([P, 1], F32, tag="len_f")
                    nc.vector.tensor_tensor(
                        len_f, se_f, ss_f, op=mybir.AluOpType.subtract
                    )
                    rlen = idx_pool.tile([P, 1], F32, tag="rlen")
                    nc.vector.reciprocal(rlen, len_f)
                    scale = idx_pool.tile([P, 1], F32, tag="scale")
                    nc.vector.tensor_scalar_mul(scale, rlen, 1.0 / L)
                    m1 = mask_pool.tile([P, SPM], F32, tag="m1")
                    nc.vector.tensor_scalar(
                        m1,
                        iota_f,
                        scalar1=ss_f[:, :1],
                        scalar2=scale[:, :1],
                        op0=mybir.AluOpType.is_ge,
                        op1=mybir.AluOpType.mult,
                    )
                    m2 = mask_pool.tile([P, SPM], F32, tag="m2")
                    nc.vector.tensor_scalar(
                        m2,
                        iota_f,
                        scalar1=se_f[:, :1],
                        scalar2=None,
                        op0=mybir.AluOpType.is_lt,
                    )
                    mM = mask_pool.tile([P, SPM], F32, tag="mM")
                    nc.vector.tensor_tensor(mM, m1, m2, op=mybir.AluOpType.mult)
                    for j in range(3):
                        ptr = ptr_pool.tile([P, P], F32, space="PSUM", tag="ptr")
                        nc.tensor.transpose(
                            ptr, mM[:, j * P : (j + 1) * P], identity
                        )
                        col = (j * WT + wt) * P
                        nc.scalar.copy(maskT_full[:, col : col + P], ptr)
                    if T:
                        ptrT = ptr_pool.tile([T, P], F32, space="PSUM", tag="ptrT")
                        nc.tensor.transpose(ptrT, mM[:, SFULL:SPM], identity)
                        nc.scalar.copy(
                            maskT_tail[:, wt * P : (wt + 1) * P], ptrT
                        )

                # --- word-embedding gather lands directly in the row tile ---
                obufs = []
                for wt in range(WT):
                    obuf = o_pool.tile(
                        [P, E + H], F32, tag="obuf", name=f"obuf_{b}_{wt}"
                    )
                    nc.gpsimd.indirect_dma_start(
                        out=obuf[:, 0:E],
                        out_offset=None,
                        in_=emb[:, :],
                        in_offset=bass.IndirectOffsetOnAxis(
                            ap=wi_list[wt][:, :1], axis=0
                        ),
                    )
                    obufs.append(obuf)

                # --- exact f32 12-layer sum: two interleaved DVE chains.
                # Tail adds go first per layer (their DMA landed long ago),
                # so the tail sum is final before the last big DMA lands ---
                hsum = hsum_pool.tile([P, CW], F32, tag="hsum", name=f"hs_{b}")
                if T:
                    nc.vector.tensor_tensor(
                        hsum[:, CF:CW], h_tiles[0][:, CF:CW],
                        h_tiles[1][:, CF:CW], op=mybir.AluOpType.add,
                    )
                nc.vector.tensor_tensor(
                    hsum[:, 0:CF], h_tiles[0][:, 0:CF], h_tiles[1][:, 0:CF],
                    op=mybir.AluOpType.add,
                )
                for l in range(2, L):
                    if T:
                        nc.vector.tensor_tensor(
                            hsum[:, CF:CW], hsum[:, CF:CW],
                            h_tiles[l][:, CF:CW], op=mybir.AluOpType.add,
                        )
                    nc.vector.tensor_tensor(
                        hsum[:, 0:CF], hsum[:, 0:CF], h_tiles[l][:, 0:CF],
                        op=mybir.AluOpType.add,
                    )

                # --- un-flatten the packed tail sum to [T, 768] via a DRAM
                # bounce (SBUF APs cannot regroup the partition dim) ---
                tail16 = None
                if T:
                    nc.scalar.dma_start(
                        out=tsc[b, :].rearrange("(p x) -> p x", p=P),
                        in_=hsum[:, CF:CW],
                    )
                    tail16 = tail_pool.tile([T, H], F32, tag="t16", name=f"t16_{b}")
                    nc.scalar.dma_start(
                        out=tail16,
                        in_=tsc[b, :].rearrange("(t x) -> t x", t=T),
                    )

                # --- span matmuls: tail first (start=True, fires before the
                # main stream ends), then the three full s-groups ---
                for wt in range(WT):
                    pout = pout_pool.tile(
                        [P, H], F32, space="PSUM", tag=f"pout{wt}",
                        name=f"pout{wt}_{b}",
                    )
                    if T:
                        for n0, nl in NCHUNKS:
                            nc.tensor.matmul(
                                pout[:, n0 : n0 + nl],
                                lhsT=maskT_tail[:, wt * P : (wt + 1) * P],
                                rhs=tail16[:, n0 : n0 + nl],
                                start=True,
                                stop=False,
                            )
                    for j in range(3):
                        col = (j * WT + wt) * P
                        for n0, nl in NCHUNKS:
                            nc.tensor.matmul(
                                pout[:, n0 : n0 + nl],
                                lhsT=maskT_full[:, col : col + P],
                                rhs=hsum[:, j * H + n0 : j * H + n0 + nl],
                                start=(j == 0 and not T),
                                stop=(j == 2),
                            )
                    # PSUM -> row tile on ACT; store once per w-tile
                    for n0, nl in NCHUNKS:
                        nc.scalar.copy(
                            obufs[wt][:, E + n0 : E + n0 + nl],
                            pout[:, n0 : n0 + nl],
                        )
                    store_eng = nc.sync if b == BPC - 1 else nc.scalar
                    wsl = slice(wt * P, (wt + 1) * P)
                    store_eng.dma_start(out=out[b, wsl, :], in_=obufs[wt])

    nc.compile()
    return nc


_NC = {}


def _tail_for(s_used):
    """Round the needed tail (beyond 384) up to a power of two <= 128."""
    if s_used <= SFULL:
        return 0
    t = s_used - SFULL
    p = 1
    while p < t:
        p *= 2
    return min(p, P)


def _get_program(T=16):
    if T not in _NC:
        _NC[T] = build_program(T)
    return _NC[T]


def make_in_maps(word_indices, span_start, span_end, W_embed, hidden_states, T):
    SP = SFULL + T
    emb = np.ascontiguousarray(W_embed, dtype=np.float32)
    in_maps = []
    for c in range(NCORES):
        bsl = slice(BPC * c, BPC * (c + 1))
        hsc = np.ascontiguousarray(
            hidden_states[:, bsl, :SP, :], dtype=np.float32
        ).reshape(L, BPC, SP * H)
        in_maps.append(
            {
                "word_indices": np.ascontiguousarray(
                    word_indices[bsl], dtype=np.int32
                ),
                "span_start": np.ascontiguousarray(span_start[bsl], dtype=np.int32),
                "span_end": np.ascontiguousarray(span_end[bsl], dtype=np.int32),
                "W_embed": emb,
                "hidden_states": hsc,
            }
        )
    return in_maps


def run(word_indices, span_start, span_end, W_embed, hidden_states, **run_kwargs):
    from concourse.bass_utils import run_bass_kernel_spmd

    s_used = int(np.max(np.asarray(span_end)[:, -1]))
    T = _tail_for(s_used)
    nc = _get_program(T)
    in_maps = make_in_maps(
        word_indices, span_start, span_end, W_embed, hidden_states, T
    )
    res = run_bass_kernel_spmd(nc, in_maps, core_ids=list(range(NCORES)), **run_kwargs)
    out = np.concatenate([res.results[c]["out"] for c in range(NCORES)], axis=0)
    return out, res


def kernel(word_indices, span_start, span_end, W_embed, hidden_states):
    out, _ = run(word_indices, span_start, span_end, W_embed, hidden_states)
    return out
